# revision 5
# baseline (speedup 1.0000x reference)
"""Trainium2 Bass kernel for nn_LPModel_85263690760360 (retrieval_knn).

Math: the reference computes, for 6000 queries (left/right of 3000 links),
the 75 smallest hyperboloid sqdists against all 30000 embeddings, and a
margin loss  (sum relu(D_i - topk_vals)) / (2*75*3000).

sqdist is a monotone non-increasing function of the Minkowski product
p = -q0*e0 + q[1:]. e[1:], and is clamped: every candidate with
p >= -(1+EPS) gets exactly sqdist m = arccosh(1+EPS)^2.  Whenever a query
has >= 75 DISTINCT nodes at the clamp, its top-75 values are all exactly m
and its loss contribution collapses to D_i - m (D_i >= GAMMA=1 > m).

This version fuses everything into ONE small fp8 transfer (0.73 MB total
vs the 4 MB of the first working version; the axon tunnel is latency +
payload bound, so wire bytes dominate the wall clock):
  - pair-aligned sharding: core c owns pairs [c*375,(c+1)*375); its
    [128 x 750] fp8 wire tensor holds the 375 left queries then the 375
    right queries.  The kernel memsets a [128 x 768] SBUF tile and DMAs
    the two blocks to columns 0..374 and 384..758 (pad columns stay zero
    and are excluded on host).
  - the certificate candidate subset IS the core's own 750 query columns
    (embedding rows are valid candidate nodes).  A subset count is a lower
    bound on the global count after subtracting the 18 zero-pad columns
    (p(q,0)=0 >= -(1+EPS), always counted) and the core's duplicate node
    count (computed exactly on host with np.unique).
  - the pair Minkowski dots fall out of the SAME matmul: with Qneg = Q^T
    with partition-0 (time dim) negated, P = Q^T.T @ Qneg gives
    p(q_i, cand_j) for all (i,j); pair i's dot is the shifted diagonal
    P[i, 384+i], extracted with an identity-mask multiply + free-axis
    accumulate on DVE.  The f32 arccosh chain then yields the pair sqdists.

Device per core: 12 fp8 matmuls [128q x 128]x[128 x 384c] -> PSUM, a
Sign(p+THR) ACT count per psum tile (free-axis accum), 3 diagonal
extractions, and the arccosh chain on [128, 3].  fp8 operands add ~1.5e-3
relative error to the final loss (tolerance 2e-2); the per-query count
flip vs f32 is <= 12, far inside the 32-count certificate margin.

Host: two-thread fp8 pack (round-to-nearest bf16 shift trick + a 64K LUT,
~2x faster than ml_dtypes' direct cast) into one buffer reused across
calls, AOT-compiled shard_map dispatch, count-gate check + closed-form
assembly  loss = mean(pair sqdist) + GAMMA - m;  exact numpy fallback if
the gate ever fails (makes kernel() total for any input).

Environment notes (this walrus/axon build):
  - walrus rejects >1 sync-wait per instruction ("Too many sync wait
    commands"): _SplitDrainTileContext splits the Tile kernel-tail drain
    into single-wait drains, and _split_multiwait() post-processes any
    remaining multi-wait instruction the same way.
  - there is no NTFF profile hook, so exec_time_ns is unavailable; the
    runner caches the jitted shard_map callable so repeat calls cost only
    host prep + transfer + dispatch through the axon tunnel.
"""
import numpy as np
import ml_dtypes
from contextlib import ExitStack

import concourse.bass as bass
import concourse.tile as tile
from concourse import mybir

F32 = mybir.dt.float32
BF16 = mybir.dt.bfloat16
F8 = mybir.dt.float8e4
F8NP = ml_dtypes.float8_e4m3

N_NODES = 30000
DIM = 128
T_LINKS = 3000
K_NEG = 75
GAMMA = 1.0
EPS = 1e-7
MAX_SQDIST = 50.0

NCORES = 8
PAIRS = T_LINKS // NCORES         # 375 pairs per core
QW = 768                          # query columns per core: 375 L, pad, 375 R
ROFF = 384                        # column offset of the right-query block
MT = 6                            # stationary 128-query tiles
NH = 2                            # moving halves per stationary tile
NHW = QW // NH                    # 384 candidate columns per half
NCNT = MT * NH                    # 12 count columns in the result
NRES = NCNT + 3                   # + 3 pair-sqdist columns

THR = np.float32(1.0 + EPS)                        # theta clip point
M_CONST = float(np.arccosh(np.float64(THR)) ** 2)  # collapsed top-k value
PAD_CANDS = QW - 2 * PAIRS                         # 18 zero candidate cols
CERT_MARGIN = 32                                   # fp8 count-flip headroom

LAST_EXEC_NS = None


class _SplitDrainTileContext(tile.TileContext):
    """TileContext whose kernel-tail drain is split into single-wait drains.

    This walrus build caps the number of sync-wait commands one instruction
    may carry; the stock tail drain waits on every active proc at once (one
    wait per engine/DMA-queue semaphore) and is rejected with "Too many sync
    wait commands".  A ladder of SP drains with one wait each executes
    sequentially on SP and is equivalent.
    """

    def _drain_and_barrier(self, tick_clock, wait_clock):
        from concourse.vector_clock import ScopedClock, VectorClock
        from concourse.tile_sem_assignment import N_PROCS

        gc = tick_clock.global_clock
        for p in range(N_PROCS):
            t = gc.peek_next(p) - 1
            if t <= 0:
                continue
            part = VectorClock([t if q == p else 0 for q in range(N_PROCS)])
            d = self.nc.sync.drain()
            wait_clock.add_sem_waits(d.ins, ScopedClock({None: part}))
        self.nc.all_engine_barrier()
        popped = self.nc._tile_sem_poison_stack.pop()
        assert popped is self._sem_poison
        self.nc.clear_and_free_semaphores(list(self.sems.allocated().values()))
        self.nc.all_engine_barrier()


def _split_multiwait(nc):
    """Split multi-wait instructions into single-wait same-engine drains.

    The walrus build in this environment rejects instructions carrying more
    than one sync-wait command ("Too many sync wait commands").  Engine
    queues execute in order, so waiting on A at queue slot n and on B at
    slot n+1 is equivalent to waiting on {A, B} at slot n+1: move all but
    the last wait onto fresh Drain instructions inserted just before the
    offender on the same engine.
    """
    import copy as _copy

    fn = nc.m.functions[0]
    template = None
    for b in fn.blocks:
        for j in b.instructions:
            if type(j).__name__ == "InstDrain":
                template = j
                break
        if template is not None:
            break
    if template is None:
        return 0
    n_split = 0
    for b in fn.blocks:
        insts = b.instructions
        idx = 0
        while idx < len(insts):
            i = insts[idx]
            si = i.sync_info
            if si is not None and si.on_wait and len(si.on_wait) > 1:
                waits = list(si.on_wait)
                for k, w in enumerate(waits[:-1]):
                    nd = _copy.deepcopy(template)
                    nd.name = f"{i.name}-wsplit{k}"
                    nd.engine = i.engine
                    nsi = nd.sync_info
                    nsi.on_wait = [w]
                    nsi.on_update = []
                    nd.sync_info = nsi
                    insts.insert(idx, nd)
                    idx += 1
                si.on_wait = [waits[-1]]
                i.sync_info = si
                n_split += 1
            idx += 1
    return n_split


def _build_nc():
    nc = bass.Bass()

    def reg_const(value):
        t = nc.alloc_sbuf_tensor(f"const-f32-{value}", [128, 1], F32)
        nc.gpsimd.memset(t.ap(), value)
        nc.const_aps.aps[(F32, float(value))] = t.ap()

    reg_const(float(THR))
    reg_const(-1.0)
    nc.all_engine_barrier()

    # per-core fp8 input: Q^T [dim x query-col]; cols 0..374 the left
    # queries, 375..749 the right queries (no padding on the wire; the
    # kernel memsets the SBUF tile and DMAs into the two column ranges)
    q = nc.dram_tensor("q", [128, 2 * PAIRS], F8, kind="ExternalInput")

    # counts (cols 0..11, ACT sign-sums) and pair sqdists (cols 12..14)
    res = nc.dram_tensor("res", [128, NRES], F32, kind="ExternalOutput")

    with _SplitDrainTileContext(nc) as tc, ExitStack() as ctx:
        weights = ctx.enter_context(tc.tile_pool(name="weights", bufs=1))
        persist = ctx.enter_context(tc.tile_pool(name="persist", bufs=1))
        dpath = ctx.enter_context(tc.tile_pool(name="dpath", bufs=1))
        scratch = ctx.enter_context(tc.tile_pool(name="scratch", bufs=3))
        psA = ctx.enter_context(tc.tile_pool(name="psA", bufs=2, space="PSUM"))
        psD = ctx.enter_context(tc.tile_pool(name="psD", bufs=2, space="PSUM"))

        q_t = weights.tile([128, QW], F8)
        nc.gpsimd.memset(q_t[:, :], 0.0)
        nc.sync.dma_start(out=q_t[:, 0:PAIRS], in_=q[:, 0:PAIRS])
        nc.sync.dma_start(out=q_t[:, ROFF:ROFF + PAIRS],
                          in_=q[:, PAIRS:2 * PAIRS])

        # Qneg = Q^T with the Minkowski time-dim (partition 0) negated, so
        # matmul yields p(q_i, cand_j) = -qi0*qj0 + <qi',qj'> directly.
        qneg = persist.tile([128, QW], F8)
        nc.scalar.activation(out=qneg[:, :], in_=q_t[:, :],
                             func=mybir.ActivationFunctionType.Copy)
        # partition offsets != 0 are illegal engine APs, so overwrite the
        # time-dim row (partition 0) with its negation in a second pass
        nc.scalar.activation(out=qneg[0:1, :], in_=q_t[0:1, :],
                             func=mybir.ActivationFunctionType.Copy,
                             scale=-1.0)

        # 128x128 identity mask for shifted-diagonal extraction
        ident = persist.tile([128, 128], F32)
        nc.gpsimd.memset(ident[:, :], 1.0)
        nc.gpsimd.affine_select(out=ident[:, :], in_=ident[:, :],
                                pattern=[[1, 128]], base=0,
                                channel_multiplier=-1,
                                compare_op=mybir.AluOpType.is_equal,
                                fill=0.0)

        a_out = persist.tile([128, NRES], F32, name="res", tag="res")
        d_t = dpath.tile([128, 3], F32)

        # certificate matmuls; the pair Minkowski dots are the shifted
        # diagonals of the n=1 halves of the first three stationary tiles
        for m in range(MT):
            w = q_t[:, m * 128:(m + 1) * 128]
            for n in range(NH):
                if (m * NH + n) % 2 == 0:
                    p_ps = psA.tile([128, NHW], F32, name="pa", tag="pa")
                else:
                    p_ps = psD.tile([128, NHW], F32, name="pd", tag="pd")
                nc.tensor.matmul(p_ps, w, qneg[:, n * NHW:(n + 1) * NHW],
                                 start=True, stop=True)
                # clip-count: sign(p + THR) summed along the free axis (ACT;
                # the DVE tensor_scalar accum_out path is broken on this HW)
                sg = scratch.tile([128, NHW], BF16, tag="sg")
                col = m * NH + n
                nc.scalar.activation(
                    out=sg, in_=p_ps,
                    func=mybir.ActivationFunctionType.Sign,
                    bias=float(THR), scale=1.0,
                    accum_out=a_out[:, col:col + 1],
                )
                if n == 1 and m < 3:
                    junk = scratch.tile([128, 128], F32, tag="dj")
                    nc.vector.scalar_tensor_tensor(
                        out=junk, in0=p_ps[:, m * 128:(m + 1) * 128],
                        scalar=1.0, in1=ident,
                        op0=mybir.AluOpType.mult, op1=mybir.AluOpType.mult,
                        accum_out=d_t[:, m:m + 1],
                    )

        # ---------------- pair sqdist: f32 arccosh chain ----------------
        th = dpath.tile([128, 3], F32)
        nc.vector.tensor_scalar(out=th, in0=d_t, scalar1=-1.0, scalar2=float(THR),
                                op0=mybir.AluOpType.mult, op1=mybir.AluOpType.max)
        th2 = dpath.tile([128, 3], F32)
        nc.scalar.activation(out=th2, in_=th, func=mybir.ActivationFunctionType.Square)
        s_t = dpath.tile([128, 3], F32)
        nc.scalar.activation(out=s_t, in_=th2,
                             func=mybir.ActivationFunctionType.Sqrt, bias=-1.0)
        # Newton refine sqrt: s <- 0.5*(s + y/s), y = th2-1
        y_t = dpath.tile([128, 3], F32)
        nc.vector.tensor_scalar(out=y_t, in0=th2, scalar1=-1.0, scalar2=None,
                                op0=mybir.AluOpType.add)
        r_t = dpath.tile([128, 3], F32)
        nc.vector.reciprocal(out=r_t, in_=s_t)
        t1 = dpath.tile([128, 3], F32)
        nc.vector.tensor_mul(out=t1, in0=y_t, in1=r_t)
        s2 = dpath.tile([128, 3], F32)
        nc.vector.tensor_add(out=s2, in0=s_t, in1=t1)
        s3 = dpath.tile([128, 3], F32)
        nc.vector.tensor_scalar(out=s3, in0=s2, scalar1=0.5, scalar2=None,
                                op0=mybir.AluOpType.mult)
        u_t = dpath.tile([128, 3], F32)
        nc.vector.tensor_add(out=u_t, in0=th, in1=s3)
        a_t = dpath.tile([128, 3], F32)
        nc.scalar.activation(out=a_t, in_=u_t, func=mybir.ActivationFunctionType.Ln)
        a2 = dpath.tile([128, 3], F32)
        nc.scalar.activation(out=a2, in_=a_t, func=mybir.ActivationFunctionType.Square)
        nc.vector.tensor_scalar(out=a_out[:, NCNT:], in0=a2,
                                scalar1=float(MAX_SQDIST),
                                scalar2=None, op0=mybir.AluOpType.min)

        nc.sync.dma_start(out=res[:, :], in_=a_out)
    _split_multiwait(nc)
    return nc


_RUNNER = None


def _make_runner():
    """Build nc once and return a cached callable
    (q_global[1024, QW] fp8) -> list of 8 per-core {res} float32 arrays.

    Mirrors concourse.bass_utils.run_bass_kernel_spmd's axon path
    (bass2jax.run_bass_via_pjrt) but hoists the trace/lower/jit out of the
    per-call path so repeat calls skip straight to transfer + execute.
    """
    import jax
    from jax.sharding import Mesh, NamedSharding, PartitionSpec
    from jax.experimental.shard_map import shard_map
    from concourse import bass2jax

    nc = _build_nc()
    bass2jax.install_neuronx_cc_hook()

    partition_name = (nc.partition_id_tensor.name
                      if nc.partition_id_tensor else None)

    in_names, out_names, out_avals, zero_outs = [], [], [], []
    for alloc in nc.m.functions[0].allocations:
        if not isinstance(alloc, mybir.MemoryLocationSet):
            continue
        name = alloc.memorylocations[0].name
        if alloc.kind == "ExternalInput":
            if name != partition_name:
                in_names.append(name)
        elif alloc.kind == "ExternalOutput":
            out_names.append(name)
            shape = tuple(alloc.tensor_shape)
            dtype = mybir.dt.np(alloc.dtype)
            out_avals.append(jax.core.ShapedArray(shape, dtype))
            zero_outs.append(np.zeros((NCORES * shape[0], *shape[1:]), dtype))
    n_params = len(in_names)
    n_outs = len(out_avals)
    all_names = list(in_names) + list(out_names)
    if partition_name is not None:
        all_names.append(partition_name)

    def _body(*args):
        operands = list(args)
        if partition_name is not None:
            operands.append(bass2jax.partition_id_tensor())
        outs = bass2jax._bass_exec_p.bind(
            *operands,
            out_avals=tuple(out_avals),
            in_names=tuple(all_names),
            out_names=tuple(out_names),
            lowering_input_output_aliases=(),
            sim_require_finite=True,
            sim_require_nnan=True,
            nc=nc,
        )
        return tuple(outs)

    devices = jax.devices()[:NCORES]
    assert len(devices) == NCORES
    mesh = Mesh(np.asarray(devices), ("core",))
    spec = PartitionSpec("core")
    in_specs = (spec,) * (n_params + n_outs)
    out_specs = (spec,) * n_outs
    # No donation: both outputs are fully written by the kernel, so the
    # pre-zeroed "output parameter" buffers never need refreshing - keep
    # them resident on device and reuse across calls (saves per-call
    # upload + donation bookkeeping).
    sharded = jax.jit(
        shard_map(_body, mesh=mesh, in_specs=in_specs, out_specs=out_specs,
                  check_rep=False),
        keep_unused=True,
    )
    ns = NamedSharding(mesh, spec)
    zdev = [jax.device_put(z, ns) for z in zero_outs]
    jax.block_until_ready(zdev)

    name_to_pos = {n: i for i, n in enumerate(in_names)}
    assert n_params == 1 and in_names[0] == "q"

    # AOT-compile once so per-call dispatch skips the jit cache machinery
    in_sds = [jax.ShapeDtypeStruct((NCORES * 128, 2 * PAIRS), F8NP,
                                   sharding=ns)]
    compiled = sharded.lower(*in_sds, *zdev).compile()

    def run(q_global):
        out_arrs = compiled(jax.device_put(q_global, ns), *zdev)
        res = []
        for c in range(NCORES):
            res.append({
                name: np.asarray(out_arrs[i]).reshape(
                    NCORES, *out_avals[i].shape)[c]
                for i, name in enumerate(out_names)
            })
        return res

    return run


def _host_fallback(emb, c, links):
    """Exact reference computation on host (safety net).

    sqdist is monotone non-increasing in the Minkowski product p, so the 75
    smallest sqdists are the 75 largest p: select them with an O(N) f32
    partition, then evaluate the arccosh chain in f64 on just those.
    Bit-identical to the full f64 sort on the reference inputs.
    """
    cs = np.float64(c[0])
    L = emb[links[:, 0]].astype(np.float64)
    R = emb[links[:, 1]].astype(np.float64)
    K = 1.0 / cs

    def sqd(prod):
        theta = np.maximum(-prod / K, 1.0 + EPS)
        return np.minimum(K * np.arccosh(theta) ** 2, MAX_SQDIST)

    d = -L[:, 0] * R[:, 0] + (L[:, 1:] * R[:, 1:]).sum(1)
    D = sqd(d) + GAMMA
    t = links.shape[0]
    embp32 = emb.copy()
    embp32[:, 0] = -embp32[:, 0]
    total = 0.0
    for Q32 in (emb[links[:, 0]], emb[links[:, 1]]):
        P32 = Q32 @ embp32.T                                   # (t, N)
        topp = -np.partition(-P32, K_NEG - 1, axis=1)[:, :K_NEG]
        S = sqd(topp.astype(np.float64))
        total += np.maximum(D[:, None] - S, 0.0).sum()
    return np.float32(total / (2.0 * K_NEG * t))


_QBUF = None
_F8_LUT = None


def _to_fp8(x):
    """f32 -> fp8 e4m3 via round-to-nearest bf16 then a 64K uint8 LUT.

    ~2x faster than ml_dtypes' elementwise f32->fp8 cast.  The double
    rounding flips ~3% of elements by one fp8 ulp vs the direct cast --
    noise relative to the fp8 quantization error itself, which the
    certificate margin and the 2e-2 loss tolerance absorb with ~10x room.
    """
    global _F8_LUT
    if _F8_LUT is None:
        _F8_LUT = (np.arange(65536, dtype=np.uint16)
                   .view(ml_dtypes.bfloat16).astype(F8NP).view(np.uint8))
    u = x.view(np.uint32)
    t = u >> 16                    # in-place chain: bf16 round-to-even
    t &= 1
    t += 0x7FFF
    t += u
    t >>= 16
    return _F8_LUT[t].view(F8NP)


def kernel(embeddings, c, train_links):
    global _RUNNER, _QBUF, LAST_EXEC_NS
    emb = np.asarray(embeddings, dtype=np.float32)
    cc = np.asarray(c, dtype=np.float32)
    links = np.asarray(train_links)

    if (abs(float(cc[0]) - 1.0) > 1e-12 or links.shape != (T_LINKS, 2)
            or emb.shape != (N_NODES, DIM)):
        return _host_fallback(emb, cc, links)

    # ---- host-side pack: fp8 Q^T per core (pair-aligned sharding, no
    # padding on the wire; numpy holds the GIL so threading doesn't help)
    if _QBUF is None:
        _QBUF = np.empty((NCORES, 128, 2 * PAIRS), F8NP)
    qb = _QBUF
    for col0, side in ((0, 0), (PAIRS, 1)):
        x = _to_fp8(emb[links[:, side]])
        qb[:, :, col0:col0 + PAIRS] = (
            x.reshape(NCORES, PAIRS, DIM).transpose(0, 2, 1))
    q_global = qb.reshape(NCORES * 128, 2 * PAIRS)

    try:
        if _RUNNER is None:
            _RUNNER = _make_runner()
        results = _RUNNER(q_global)
    except Exception:
        return _host_fallback(emb, cc, links)
    LAST_EXEC_NS = None

    # ---- unshard / certificate gate / closed-form assembly
    sq_sum = 0.0
    ok = True
    for core in range(NCORES):
        r = results[core]["res"].astype(np.float64)
        s = r[:, :NCNT]
        # sign-sum to count over both 384-col halves
        cnt = (float(QW) + s[:, 0::2] + s[:, 1::2]) / 2.0     # (128, 6)
        cflat = cnt.T.reshape(-1)                             # index m*128+p
        cl = links[core * PAIRS:(core + 1) * PAIRS]
        dup = 2 * PAIRS - np.unique(cl).size
        need = K_NEG + PAD_CANDS + dup + CERT_MARGIN
        valid_min = min(cflat[0:PAIRS].min(), cflat[ROFF:ROFF + PAIRS].min())
        if valid_min < need:
            ok = False
            break
        sq = r[:, NCNT:].T.reshape(-1)[:PAIRS]
        sq_sum += sq.sum()

    if not ok:
        # top-k collapse not certified for some query -> exact fallback
        return _host_fallback(emb, cc, links)

    loss = sq_sum / T_LINKS + GAMMA - M_CONST  # mean(D) - m
    return np.float32(loss)


# revision 7
# speedup vs baseline: 1.0296x; 1.0296x over previous
"""Trainium2 Bass kernel for nn_LPModel_85263690760360 (retrieval_knn).

Math: the reference computes, for 6000 queries (left/right of 3000 links),
the 75 smallest hyperboloid sqdists against all 30000 embeddings, and a
margin loss  (sum relu(D_i - topk_vals)) / (2*75*3000).

sqdist is a monotone non-increasing function of the Minkowski product
p = -q0*e0 + q[1:]. e[1:], and is clamped: every candidate with
p >= -(1+EPS) gets exactly sqdist m = arccosh(1+EPS)^2.  Whenever a query
has >= 75 DISTINCT nodes at the clamp, its top-75 values are all exactly m
and its loss contribution collapses to D_i - m (D_i >= GAMMA=1 > m).

This version fuses everything into ONE small fp8 transfer (0.73 MB total
vs the 4 MB of the first working version; the axon tunnel is latency +
payload bound, so wire bytes dominate the wall clock):
  - pair-aligned sharding: core c owns pairs [c*375,(c+1)*375); its
    [750 x 128] fp8 wire tensor holds the 375 left query ROWS then the
    375 right query rows (natural gather order, so the host pack is one
    fused gather + LUT with no strided store).  The kernel loads the six
    128-row blocks side by side and transposes them on the TensorEngine
    (bf16 identity transpose -> PSUM -> SBUF) to build Q^T.
  - the certificate candidate subset IS the core's own 750 query columns
    (embedding rows are valid candidate nodes).  A subset count is a lower
    bound on the global count after subtracting the core's duplicate node
    count (computed exactly on host with np.unique).
  - the pair Minkowski dots fall out of the SAME matmul: with Qneg = Q^T
    with partition-0 (time dim) negated, P = Q^T.T @ Qneg gives
    p(q_i, cand_j) for all (i,j); pair i's dot is the shifted diagonal
    P[i, 384+i], extracted with an identity-mask multiply + free-axis
    accumulate on DVE.  The f32 arccosh chain then yields the pair sqdists.

Device per core: 6 TensorE transposes, 12 bf16 matmuls (operands carry
fp8-quantized values) [128q x 128]x[128 x 375c] -> PSUM, a Sign(p+THR)
ACT count per psum tile (free-axis accum), 3 diagonal extractions, and
the arccosh chain on [128, 3].  fp8 operands add ~1.5e-3 relative error
to the final loss (tolerance 2e-2); the per-query count flip vs f32 is
<= 12, far inside the 32-count certificate margin.

Host: one fused wire-order gather + fp8 LUT cast (round-to-nearest bf16
shift trick + a 64K uint8 LUT, ~2x faster than ml_dtypes' direct cast),
AOT-compiled shard_map dispatch, count-gate check + closed-form assembly
loss = mean(pair sqdist) + GAMMA - m;  exact numpy fallback if the gate
ever fails (makes kernel() total for any input).

Environment notes (this walrus/axon build):
  - walrus rejects >1 sync-wait per instruction ("Too many sync wait
    commands"): _SplitDrainTileContext splits the Tile kernel-tail drain
    into single-wait drains, and _split_multiwait() post-processes any
    remaining multi-wait instruction the same way.
  - there is no NTFF profile hook, so exec_time_ns is unavailable; the
    runner caches the jitted shard_map callable so repeat calls cost only
    host prep + transfer + dispatch through the axon tunnel.
"""
import numpy as np
import ml_dtypes
from contextlib import ExitStack

import concourse.bass as bass
import concourse.tile as tile
from concourse import mybir

F32 = mybir.dt.float32
BF16 = mybir.dt.bfloat16
F8 = mybir.dt.float8e4
F8NP = ml_dtypes.float8_e4m3

N_NODES = 30000
DIM = 128
T_LINKS = 3000
K_NEG = 75
GAMMA = 1.0
EPS = 1e-7
MAX_SQDIST = 50.0

NCORES = 8
PAIRS = T_LINKS // NCORES         # 375 pairs per core
QW = 2 * PAIRS                    # 750 query columns: 375 L then 375 R
ROFF = PAIRS                      # column offset of the right-query block
MT = 6                            # stationary query tiles (last is 110 wide)
NH = 2                            # moving halves per stationary tile
NHW = QW // NH                    # 375 candidate columns per half
NCNT = MT * NH                    # 12 count columns in the result
NRES = NCNT + 3                   # + 3 pair-sqdist columns

THR = np.float32(1.0 + EPS)                        # theta clip point
M_CONST = float(np.arccosh(np.float64(THR)) ** 2)  # collapsed top-k value
CERT_MARGIN = 32                                   # fp8 count-flip headroom

LAST_EXEC_NS = None


class _SplitDrainTileContext(tile.TileContext):
    """TileContext whose kernel-tail drain is split into single-wait drains.

    This walrus build caps the number of sync-wait commands one instruction
    may carry; the stock tail drain waits on every active proc at once (one
    wait per engine/DMA-queue semaphore) and is rejected with "Too many sync
    wait commands".  A ladder of SP drains with one wait each executes
    sequentially on SP and is equivalent.
    """

    def _drain_and_barrier(self, tick_clock, wait_clock):
        from concourse.vector_clock import ScopedClock, VectorClock
        from concourse.tile_sem_assignment import N_PROCS

        gc = tick_clock.global_clock
        for p in range(N_PROCS):
            t = gc.peek_next(p) - 1
            if t <= 0:
                continue
            part = VectorClock([t if q == p else 0 for q in range(N_PROCS)])
            d = self.nc.sync.drain()
            wait_clock.add_sem_waits(d.ins, ScopedClock({None: part}))
        self.nc.all_engine_barrier()
        popped = self.nc._tile_sem_poison_stack.pop()
        assert popped is self._sem_poison
        self.nc.clear_and_free_semaphores(list(self.sems.allocated().values()))
        self.nc.all_engine_barrier()


def _split_multiwait(nc):
    """Split multi-wait instructions into single-wait same-engine drains.

    The walrus build in this environment rejects instructions carrying more
    than one sync-wait command ("Too many sync wait commands").  Engine
    queues execute in order, so waiting on A at queue slot n and on B at
    slot n+1 is equivalent to waiting on {A, B} at slot n+1: move all but
    the last wait onto fresh Drain instructions inserted just before the
    offender on the same engine.
    """
    import copy as _copy

    fn = nc.m.functions[0]
    template = None
    for b in fn.blocks:
        for j in b.instructions:
            if type(j).__name__ == "InstDrain":
                template = j
                break
        if template is not None:
            break
    if template is None:
        return 0
    n_split = 0
    for b in fn.blocks:
        insts = b.instructions
        idx = 0
        while idx < len(insts):
            i = insts[idx]
            si = i.sync_info
            if si is not None and si.on_wait and len(si.on_wait) > 1:
                waits = list(si.on_wait)
                for k, w in enumerate(waits[:-1]):
                    nd = _copy.deepcopy(template)
                    nd.name = f"{i.name}-wsplit{k}"
                    nd.engine = i.engine
                    nsi = nd.sync_info
                    nsi.on_wait = [w]
                    nsi.on_update = []
                    nd.sync_info = nsi
                    insts.insert(idx, nd)
                    idx += 1
                si.on_wait = [waits[-1]]
                i.sync_info = si
                n_split += 1
            idx += 1
    return n_split


def _build_nc():
    nc = bass.Bass()

    def reg_const(value):
        t = nc.alloc_sbuf_tensor(f"const-f32-{value}", [128, 1], F32)
        nc.gpsimd.memset(t.ap(), value)
        nc.const_aps.aps[(F32, float(value))] = t.ap()

    reg_const(float(THR))
    reg_const(-1.0)
    nc.all_engine_barrier()

    # per-core fp8 input in NATURAL row layout (query rows x dims): rows
    # 0..374 the left queries, 375..749 the right queries.  The host pack
    # is then a single fused gather+LUT with no transposed store; the
    # kernel transposes on the TensorEngine.
    q = nc.dram_tensor("q", [QW, DIM], F8, kind="ExternalInput")

    # counts (cols 0..11, ACT sign-sums) and pair sqdists (cols 12..14)
    res = nc.dram_tensor("res", [128, NRES], F32, kind="ExternalOutput")

    with _SplitDrainTileContext(nc) as tc, ExitStack() as ctx:
        weights = ctx.enter_context(tc.tile_pool(name="weights", bufs=1))
        persist = ctx.enter_context(tc.tile_pool(name="persist", bufs=1))
        dpath = ctx.enter_context(tc.tile_pool(name="dpath", bufs=1))
        scratch = ctx.enter_context(tc.tile_pool(name="scratch", bufs=3))
        psA = ctx.enter_context(tc.tile_pool(name="psA", bufs=2, space="PSUM"))
        psD = ctx.enter_context(tc.tile_pool(name="psD", bufs=2, space="PSUM"))
        psT = ctx.enter_context(tc.tile_pool(name="psT", bufs=2, space="PSUM"))

        # row-block load: qrows[:, t*128 + k] = q[t*128 + p, k]; the last
        # block holds rows 640..749 in partitions 0..109 (tail memset to 0)
        qrows = weights.tile([128, MT * 128], F8)
        nc.gpsimd.memset(qrows[:, (MT - 1) * 128:], 0.0)
        for t in range(MT):
            rows = min(128, QW - t * 128)
            nc.sync.dma_start(out=qrows[0:rows, t * 128:(t + 1) * 128],
                              in_=q[t * 128:t * 128 + rows, :])
        qbf = weights.tile([128, MT * 128], BF16)
        nc.scalar.activation(out=qbf, in_=qrows,
                             func=mybir.ActivationFunctionType.Copy)

        # 128x128 identities: bf16 for the TensorE transposes, f32 as the
        # shifted-diagonal extraction mask
        identb = persist.tile([128, 128], BF16)
        nc.gpsimd.memset(identb[:, :], 1.0)
        nc.gpsimd.affine_select(out=identb[:, :], in_=identb[:, :],
                                pattern=[[1, 128]], base=0,
                                channel_multiplier=-1,
                                compare_op=mybir.AluOpType.is_equal,
                                fill=0.0)
        ident = persist.tile([128, 128], F32)
        nc.gpsimd.memset(ident[:, :], 1.0)
        nc.gpsimd.affine_select(out=ident[:, :], in_=ident[:, :],
                                pattern=[[1, 128]], base=0,
                                channel_multiplier=-1,
                                compare_op=mybir.AluOpType.is_equal,
                                fill=0.0)

        # Q^T via TensorE transpose of each row block
        q_t = weights.tile([128, QW], BF16)
        for t in range(MT):
            cols = min(128, QW - t * 128)
            pt = psT.tile([128, 128], BF16, name="pt", tag="pt")
            nc.tensor.transpose(pt, qbf[:, t * 128:(t + 1) * 128], identb)
            nc.scalar.activation(out=q_t[:, t * 128:t * 128 + cols],
                                 in_=pt[:, 0:cols],
                                 func=mybir.ActivationFunctionType.Copy)

        # Qneg = Q^T with the Minkowski time-dim (partition 0) negated, so
        # matmul yields p(q_i, cand_j) = -qi0*qj0 + <qi',qj'> directly.
        qneg = persist.tile([128, QW], BF16)
        nc.scalar.activation(out=qneg[:, :], in_=q_t[:, :],
                             func=mybir.ActivationFunctionType.Copy)
        # partition offsets != 0 are illegal engine APs, so overwrite the
        # time-dim row (partition 0) with its negation in a second pass
        nc.scalar.activation(out=qneg[0:1, :], in_=q_t[0:1, :],
                             func=mybir.ActivationFunctionType.Copy,
                             scale=-1.0)

        a_out = persist.tile([128, NRES], F32, name="res", tag="res")
        nc.gpsimd.memset(a_out[:, :], 0.0)
        d_t = dpath.tile([128, 3], F32)

        # certificate matmuls; the pair Minkowski dots are the diagonals of
        # the n=1 (right-query) halves of the first three stationary tiles
        for m in range(MT):
            mw = min(128, QW - m * 128)
            w = q_t[:, m * 128:m * 128 + mw]
            for n in range(NH):
                if (m * NH + n) % 2 == 0:
                    p_ps = psA.tile([128, NHW], F32, name="pa", tag="pa")
                else:
                    p_ps = psD.tile([128, NHW], F32, name="pd", tag="pd")
                nc.tensor.matmul(p_ps[0:mw, :], w,
                                 qneg[:, n * NHW:(n + 1) * NHW],
                                 start=True, stop=True)
                # clip-count: sign(p + THR) summed along the free axis (ACT;
                # the DVE tensor_scalar accum_out path is broken on this HW)
                sg = scratch.tile([128, NHW], BF16, tag="sg")
                col = m * NH + n
                nc.scalar.activation(
                    out=sg[0:mw, :], in_=p_ps[0:mw, :],
                    func=mybir.ActivationFunctionType.Sign,
                    bias=float(THR), scale=1.0,
                    accum_out=a_out[0:mw, col:col + 1],
                )
                if n == 1 and m < 3:
                    dw = min(128, NHW - m * 128)     # 128,128,119
                    junk = scratch.tile([128, 128], F32, tag="dj")
                    nc.vector.scalar_tensor_tensor(
                        out=junk[:, 0:dw],
                        in0=p_ps[:, m * 128:m * 128 + dw],
                        scalar=1.0, in1=ident[:, 0:dw],
                        op0=mybir.AluOpType.mult, op1=mybir.AluOpType.mult,
                        accum_out=d_t[:, m:m + 1],
                    )

        # ---------------- pair sqdist: f32 arccosh chain ----------------
        th = dpath.tile([128, 3], F32)
        nc.vector.tensor_scalar(out=th, in0=d_t, scalar1=-1.0, scalar2=float(THR),
                                op0=mybir.AluOpType.mult, op1=mybir.AluOpType.max)
        th2 = dpath.tile([128, 3], F32)
        nc.scalar.activation(out=th2, in_=th, func=mybir.ActivationFunctionType.Square)
        s_t = dpath.tile([128, 3], F32)
        nc.scalar.activation(out=s_t, in_=th2,
                             func=mybir.ActivationFunctionType.Sqrt, bias=-1.0)
        # Newton refine sqrt: s <- 0.5*(s + y/s), y = th2-1
        y_t = dpath.tile([128, 3], F32)
        nc.vector.tensor_scalar(out=y_t, in0=th2, scalar1=-1.0, scalar2=None,
                                op0=mybir.AluOpType.add)
        r_t = dpath.tile([128, 3], F32)
        nc.vector.reciprocal(out=r_t, in_=s_t)
        t1 = dpath.tile([128, 3], F32)
        nc.vector.tensor_mul(out=t1, in0=y_t, in1=r_t)
        s2 = dpath.tile([128, 3], F32)
        nc.vector.tensor_add(out=s2, in0=s_t, in1=t1)
        s3 = dpath.tile([128, 3], F32)
        nc.vector.tensor_scalar(out=s3, in0=s2, scalar1=0.5, scalar2=None,
                                op0=mybir.AluOpType.mult)
        u_t = dpath.tile([128, 3], F32)
        nc.vector.tensor_add(out=u_t, in0=th, in1=s3)
        a_t = dpath.tile([128, 3], F32)
        nc.scalar.activation(out=a_t, in_=u_t, func=mybir.ActivationFunctionType.Ln)
        a2 = dpath.tile([128, 3], F32)
        nc.scalar.activation(out=a2, in_=a_t, func=mybir.ActivationFunctionType.Square)
        nc.vector.tensor_scalar(out=a_out[:, NCNT:], in0=a2,
                                scalar1=float(MAX_SQDIST),
                                scalar2=None, op0=mybir.AluOpType.min)

        nc.sync.dma_start(out=res[:, :], in_=a_out)
    _split_multiwait(nc)
    return nc


_RUNNER = None


def _make_runner():
    """Build nc once and return a cached callable
    (q_global[1024, QW] fp8) -> list of 8 per-core {res} float32 arrays.

    Mirrors concourse.bass_utils.run_bass_kernel_spmd's axon path
    (bass2jax.run_bass_via_pjrt) but hoists the trace/lower/jit out of the
    per-call path so repeat calls skip straight to transfer + execute.
    """
    import jax
    from jax.sharding import Mesh, NamedSharding, PartitionSpec
    from jax.experimental.shard_map import shard_map
    from concourse import bass2jax

    nc = _build_nc()
    bass2jax.install_neuronx_cc_hook()

    partition_name = (nc.partition_id_tensor.name
                      if nc.partition_id_tensor else None)

    in_names, out_names, out_avals, zero_outs = [], [], [], []
    for alloc in nc.m.functions[0].allocations:
        if not isinstance(alloc, mybir.MemoryLocationSet):
            continue
        name = alloc.memorylocations[0].name
        if alloc.kind == "ExternalInput":
            if name != partition_name:
                in_names.append(name)
        elif alloc.kind == "ExternalOutput":
            out_names.append(name)
            shape = tuple(alloc.tensor_shape)
            dtype = mybir.dt.np(alloc.dtype)
            out_avals.append(jax.core.ShapedArray(shape, dtype))
            zero_outs.append(np.zeros((NCORES * shape[0], *shape[1:]), dtype))
    n_params = len(in_names)
    n_outs = len(out_avals)
    all_names = list(in_names) + list(out_names)
    if partition_name is not None:
        all_names.append(partition_name)

    def _body(*args):
        operands = list(args)
        if partition_name is not None:
            operands.append(bass2jax.partition_id_tensor())
        outs = bass2jax._bass_exec_p.bind(
            *operands,
            out_avals=tuple(out_avals),
            in_names=tuple(all_names),
            out_names=tuple(out_names),
            lowering_input_output_aliases=(),
            sim_require_finite=True,
            sim_require_nnan=True,
            nc=nc,
        )
        return tuple(outs)

    devices = jax.devices()[:NCORES]
    assert len(devices) == NCORES
    mesh = Mesh(np.asarray(devices), ("core",))
    spec = PartitionSpec("core")
    in_specs = (spec,) * (n_params + n_outs)
    out_specs = (spec,) * n_outs
    # No donation: both outputs are fully written by the kernel, so the
    # pre-zeroed "output parameter" buffers never need refreshing - keep
    # them resident on device and reuse across calls (saves per-call
    # upload + donation bookkeeping).
    sharded = jax.jit(
        shard_map(_body, mesh=mesh, in_specs=in_specs, out_specs=out_specs,
                  check_rep=False),
        keep_unused=True,
    )
    ns = NamedSharding(mesh, spec)
    zdev = [jax.device_put(z, ns) for z in zero_outs]
    jax.block_until_ready(zdev)

    name_to_pos = {n: i for i, n in enumerate(in_names)}
    assert n_params == 1 and in_names[0] == "q"

    # AOT-compile once so per-call dispatch skips the jit cache machinery
    in_sds = [jax.ShapeDtypeStruct((NCORES * QW, DIM), F8NP, sharding=ns)]
    compiled = sharded.lower(*in_sds, *zdev).compile()

    def run(q_global):
        out_arrs = compiled(jax.device_put(q_global, ns), *zdev)
        res = []
        for c in range(NCORES):
            res.append({
                name: np.asarray(out_arrs[i]).reshape(
                    NCORES, *out_avals[i].shape)[c]
                for i, name in enumerate(out_names)
            })
        return res

    return run


def _host_fallback(emb, c, links):
    """Exact reference computation on host (safety net).

    sqdist is monotone non-increasing in the Minkowski product p, so the 75
    smallest sqdists are the 75 largest p: select them with an O(N) f32
    partition, then evaluate the arccosh chain in f64 on just those.
    Bit-identical to the full f64 sort on the reference inputs.
    """
    cs = np.float64(c[0])
    L = emb[links[:, 0]].astype(np.float64)
    R = emb[links[:, 1]].astype(np.float64)
    K = 1.0 / cs

    def sqd(prod):
        theta = np.maximum(-prod / K, 1.0 + EPS)
        return np.minimum(K * np.arccosh(theta) ** 2, MAX_SQDIST)

    d = -L[:, 0] * R[:, 0] + (L[:, 1:] * R[:, 1:]).sum(1)
    D = sqd(d) + GAMMA
    t = links.shape[0]
    embp32 = emb.copy()
    embp32[:, 0] = -embp32[:, 0]
    total = 0.0
    for Q32 in (emb[links[:, 0]], emb[links[:, 1]]):
        P32 = Q32 @ embp32.T                                   # (t, N)
        topp = -np.partition(-P32, K_NEG - 1, axis=1)[:, :K_NEG]
        S = sqd(topp.astype(np.float64))
        total += np.maximum(D[:, None] - S, 0.0).sum()
    return np.float32(total / (2.0 * K_NEG * t))


_QBUF = None
_F8_LUT = None


def _to_fp8(x):
    """f32 -> fp8 e4m3 via round-to-nearest bf16 then a 64K uint8 LUT.

    ~2x faster than ml_dtypes' elementwise f32->fp8 cast.  The double
    rounding flips ~3% of elements by one fp8 ulp vs the direct cast --
    noise relative to the fp8 quantization error itself, which the
    certificate margin and the 2e-2 loss tolerance absorb with ~10x room.
    """
    global _F8_LUT
    if _F8_LUT is None:
        with np.errstate(invalid="ignore"):   # bf16 NaN patterns -> fp8 NaN
            _F8_LUT = (np.arange(65536, dtype=np.uint16)
                       .view(ml_dtypes.bfloat16).astype(F8NP).view(np.uint8))
    u = x.view(np.uint32)
    t = u >> 16                    # in-place chain: bf16 round-to-even
    t &= 1
    t += 0x7FFF
    t += u
    t >>= 16
    return _F8_LUT[t].view(F8NP)


def kernel(embeddings, c, train_links):
    global _RUNNER, _QBUF, LAST_EXEC_NS
    emb = np.asarray(embeddings, dtype=np.float32)
    cc = np.asarray(c, dtype=np.float32)
    links = np.asarray(train_links)

    if (abs(float(cc[0]) - 1.0) > 1e-12 or links.shape != (T_LINKS, 2)
            or emb.shape != (N_NODES, DIM)):
        return _host_fallback(emb, cc, links)

    # ---- host-side pack: one fused gather + fp8 LUT in wire order (core
    # c rows: its 375 left queries then its 375 right queries); the device
    # transposes on the TensorEngine, so no strided host store is needed
    idx = links.reshape(NCORES, PAIRS, 2).transpose(0, 2, 1).reshape(-1)
    q_global = _to_fp8(emb[idx])               # (6000, 128) fp8

    try:
        if _RUNNER is None:
            _RUNNER = _make_runner()
        results = _RUNNER(q_global)
    except Exception:
        return _host_fallback(emb, cc, links)
    LAST_EXEC_NS = None

    # ---- unshard / certificate gate / closed-form assembly
    sq_sum = 0.0
    ok = True
    for core in range(NCORES):
        r = results[core]["res"].astype(np.float64)
        s = r[:, :NCNT]
        # sign-sum to count over both 375-col halves
        cnt = (float(QW) + s[:, 0::2] + s[:, 1::2]) / 2.0     # (128, 6)
        cflat = cnt.T.reshape(-1)                             # index m*128+p
        cl = links[core * PAIRS:(core + 1) * PAIRS]
        dup = 2 * PAIRS - np.unique(cl).size
        need = K_NEG + dup + CERT_MARGIN
        if cflat[0:QW].min() < need:
            ok = False
            break
        sq = r[:, NCNT:].T.reshape(-1)[:PAIRS]
        sq_sum += sq.sum()

    if not ok:
        # top-k collapse not certified for some query -> exact fallback
        return _host_fallback(emb, cc, links)

    loss = sq_sum / T_LINKS + GAMMA - M_CONST  # mean(D) - m
    return np.float32(loss)
